# revision 72
# baseline (speedup 1.0000x reference)
"""Cross-attention Trainium2 kernel (8 NeuronCores, Bass/Tile).

Problem (hardcoded): B=2, SQ=SKV=2048, D=1024, H=16 heads, HD=64.
  q  = query @ Wq + bq
  kv = context @ Wkv + bkv ; split into k, v per head
  o  = softmax(q k^T / sqrt(hd) + mask) v         (mask: -inf where True)
  out = o @ Wout + bout

Sharding: core c = (b, g) with b = c // 4 (batch), g = c % 4 (head group of 4).
Each core computes its batch's attention for its 4 heads and the partial out
projection (Wout rows for those heads); host sums the 4 partials per batch and
adds bout (linearity of the out projection).

Everything on-chip runs "transposed" (feature dim on partitions, tokens on the
free dim), so the host passes query/context transposed and gets the partial
output transposed back. Softmax uses no max subtraction (scores are ~N(0,1)
here; exp is safe in fp32) and folds masking into V: v rows are scaled by
keep=1-mask and an extra "keep" column of V yields the softmax denominator via
the same PE accumulation.

Inputs and all activation tiles are bfloat16 (within the rel-err budget; the
PE streams bf16 at the same 1 col/cycle as fp32r but DMA traffic halves, which
un-bottlenecks the startup phase where ~6 MB must land before the first
attention chunk completes). PSUM accumulation, softmax reciprocal, and the
output partials stay fp32/fp32r.

NOTE (hardware-validated): PSUM accumulation groups must be emitted
contiguously per region — interleaving two open start/stop accumulation
sequences that share a PSUM bank silently corrupts results on HW (CoreSim
does not catch it).

`passes` emits the whole body N times into one NEFF (weights loaded once) —
used by test.py to measure one steady-state iteration differentially.
"""

import sys

sys.path.insert(0, "/opt/trn_rl_repo")

import numpy as np

B, SQ, SKV, D, H, HD = 2, 2048, 2048, 1024, 16, 64
HG = 4                # heads per core
COLS = HG * HD        # 256 projected columns per core (per q/k/v)
DK = D // 128         # 8 contraction tiles
SQC = 512             # sq chunk (psum bank)
NSQC = SQ // SQC
SKC = 512             # skv chunk for kv projection
NSKC = SKV // SKC
NJ = SKV // 128       # 16 skv tiles for attention

_CACHE = {}


def _build(with_bias=False, passes=1):
    import concourse.bacc as bacc
    import concourse.mybir as mybir
    import concourse.tile as tile

    F32 = mybir.dt.float32
    F32R = mybir.dt.float32r
    BF = mybir.dt.bfloat16
    EXP = mybir.ActivationFunctionType.Exp

    nc = bacc.Bacc()

    # ---- DRAM I/O (per core) ----
    qryT = nc.dram_tensor("qryT", [D, SQ], BF, kind="ExternalInput")
    ctxT = nc.dram_tensor("ctxT", [D, SKV], BF, kind="ExternalInput")
    wq = nc.dram_tensor("wq", [D, COLS], BF, kind="ExternalInput")
    wk = nc.dram_tensor("wk", [D, COLS], BF, kind="ExternalInput")
    wv = nc.dram_tensor("wv", [D, COLS], BF, kind="ExternalInput")
    wout = nc.dram_tensor("wout", [COLS, D], BF, kind="ExternalInput")
    bq = nc.dram_tensor("bq", [1, COLS], BF, kind="ExternalInput")
    bk = nc.dram_tensor("bk", [1, COLS], BF, kind="ExternalInput")
    bv = nc.dram_tensor("bv", [1, COLS], BF, kind="ExternalInput")
    ones = nc.dram_tensor("ones", [1, SQC], F32R, kind="ExternalInput")
    ones2 = nc.dram_tensor("ones2", [2, 128], F32R, kind="ExternalInput")
    ones_bf = nc.dram_tensor("ones_bf", [1, SQC], BF, kind="ExternalInput")
    keep = nc.dram_tensor("keep", [128, NJ], F32, kind="ExternalInput")
    keep_rep = nc.dram_tensor("keep_rep", [128, NJ * HG], F32, kind="ExternalInput")
    outT = nc.dram_tensor("outT", [D, SQ], F32, kind="ExternalOutput")

    with tile.TileContext(nc) as tc:
        with (
            tc.tile_pool(name="w", bufs=1) as wp,
            tc.tile_pool(name="big", bufs=1) as bigp,
            tc.tile_pool(name="strips", bufs=3) as sp,
            tc.tile_pool(name="work", bufs=1) as workp,
            tc.tile_pool(name="ps", bufs=1, space="PSUM") as psp,
        ):
            # ---- weights / constants (loaded once) ----
            wq_sb = wp.tile([128, DK, COLS], BF)
            wk_sb = wp.tile([128, DK, COLS], BF)
            wv_sb = wp.tile([128, DK, COLS], BF)
            wout_sb = wp.tile([128, 2, D], BF)
            bq_sb = wp.tile([1, COLS], BF)
            bk_sb = wp.tile([1, COLS], BF)
            bv_sb = wp.tile([1, COLS], BF)
            ones_sb = wp.tile([1, SQC], F32R)
            ones_bf_sb = wp.tile([1, SQC], BF)
            keep_sb = wp.tile([128, NJ], F32)
            keep_rep_sb = wp.tile([128, NJ, HG], F32)

            ctxT_r = ctxT.ap().rearrange("(t p) s -> p t s", p=128)
            qryT_r = qryT.ap().rearrange("(t p) s -> p t s", p=128)
            outT_r = outT.ap().rearrange("(t p) s -> p t s", p=128)
            wk_r = wk.ap().rearrange("(t p) m -> p t m", p=128)
            wq_r = wq.ap().rearrange("(t p) m -> p t m", p=128)

            # ---- persistent activations ----
            kt_sb = bigp.tile([128, 2, SKV], BF)          # k^T, head pair per 64-row band
            v_sb = bigp.tile([128, NJ, HG, HD + 1], BF)   # v + keep column, [skv%128, j, h, :]
            qt_all = bigp.tile([128, 2, SQ], BF)          # q^T for all chunks

            def run_pass(first):
                # DMA granularity: the HWDGE costs ~625ns per descriptor
                # (serialized), so each DMA should carry >=~0.25 MB. The
                # startup-critical wk/ctx0 pair is split into 3 chunks each
                # (interleaved in consumption order) so the first kT matmul
                # starts after ~0.4 MB, not 1 MB.
                ctx0_sb = sp.tile([128, DK, SKC], BF, tag="strip", name="ctx0_sb")
                for dlo, dhi in ((0, 2), (2, 5), (5, 8)):
                    if first:
                        nc.sync.dma_start(wk_sb[:, dlo:dhi, :], wk_r[:, dlo:dhi, :])
                    nc.sync.dma_start(ctx0_sb[:, dlo:dhi, :], ctxT_r[:, dlo:dhi, 0:SKC])
                # qproj(0) runs between kT-jc0 and v-jc0 on the PE, so its
                # inputs (qry0, wq) come right after the kT inputs.
                qry0_sb = sp.tile([128, DK, SQC], BF, tag="strip", name="qry0_sb")
                for dlo, dhi in ((0, 4), (4, 8)):
                    if first:
                        nc.sync.dma_start(wq_sb[:, dlo:dhi, :], wq_r[:, dlo:dhi, :])
                    nc.sync.dma_start(qry0_sb[:, dlo:dhi, :], qryT_r[:, dlo:dhi, 0:SQC])
                if first:
                    nc.sync.dma_start(wv_sb[:], wv.ap().rearrange("(t p) m -> p t m", p=128))
                    nc.sync.dma_start(keep_sb[:], keep.ap())
                    nc.sync.dma_start(
                        keep_rep_sb[:],
                        keep_rep.ap().rearrange("p (j h) -> p j h", h=HG),
                    )
                    if with_bias:
                        nc.sync.dma_start(ones_bf_sb[:], ones_bf.ap())
                        nc.sync.dma_start(bk_sb[:], bk.ap())
                        nc.sync.dma_start(bq_sb[:], bq.ap())
                        nc.sync.dma_start(bv_sb[:], bv.ap())
                # pre-issue the remaining ctx strips so they queue ahead of
                # wout and the later qry strips; the small keep_rep/ones loads
                # slot in after the strip they must not delay
                strip_tiles = [ctx0_sb]
                for jc in range(1, NSKC):
                    st = sp.tile([128, DK, SKC], BF, tag="strip", name=f"ctx{jc}_sb")
                    nc.sync.dma_start(st[:], ctxT_r[:, :, jc * SKC:(jc + 1) * SKC])
                    strip_tiles.append(st)
                    if first and jc == 2:
                        nc.sync.dma_start(ones_sb[:], ones.ap())

                if first:
                    # the keep column of v is constant: fill it for all chunks
                    # in one op
                    nc.vector.tensor_copy(v_sb[:, :, :, HD:HD + 1], keep_rep_sb[:])

                # ===== Phase K (kv projection), interleaved into head 0 =====
                def emit_K_kT(jc):
                    ctx_sb = strip_tiles[jc]
                    pk = psp.tile([128, 2, SKC], F32, tag="mm", bufs=2, name="pk")
                    for cc in range(2):
                        for d in range(DK):
                            nc.tensor.matmul(
                                pk[:, cc, :],
                                wk_sb[:, d, cc * 128:(cc + 1) * 128],
                                ctx_sb[:, d, :],
                                start=(d == 0), stop=(not with_bias and d == DK - 1),
                            )
                        if with_bias:
                            nc.tensor.matmul(
                                pk[:, cc, :],
                                bk_sb[0:1, cc * 128:(cc + 1) * 128],
                                ones_bf_sb[0:1, :],
                                start=False, stop=True,
                            )
                    nc.vector.tensor_copy(kt_sb[:, :, jc * SKC:(jc + 1) * SKC], pk[:])

                def _pv_slice(pv, jj):
                    return pv[:, jj // 2, (jj % 2) * COLS:(jj % 2) * COLS + COLS]

                def _v_finish1(pv, jc, jj, on_act=False):
                    # on_act: run the scale on the Activation engine (idle
                    # during phase K) so it overlaps the DVE kt copy — the AV
                    # group that consumes this chunk's v tiles is only ~2us out.
                    j = jc * 4 + jj
                    src = _pv_slice(pv, jj).rearrange("p (h e) -> p h e", h=HG)
                    if on_act:
                        nc.scalar.mul(v_sb[:, j, :, 0:HD], src, keep_sb[:, j:j + 1])
                    else:
                        nc.vector.tensor_scalar_mul(
                            v_sb[:, j, :, 0:HD], src, keep_sb[:, j:j + 1]
                        )

                def emit_K_v(jc, on_act=False):
                    # each jj region's start..stop matmuls stay contiguous
                    # (HW requirement); its scale is emitted right after so the
                    # Act/DVE work overlaps the next region's matmuls
                    ctx_sb = strip_tiles[jc]
                    pv = psp.tile([128, 2, SKC], F32, tag="mm", bufs=2, name="pv")
                    for jj in range(4):
                        for d in range(DK):
                            nc.tensor.matmul(
                                _pv_slice(pv, jj),
                                ctx_sb[:, d, jj * 128:(jj + 1) * 128],
                                wv_sb[:, d, :],
                                start=(d == 0), stop=(not with_bias and d == DK - 1),
                            )
                        if with_bias:
                            nc.tensor.matmul(
                                _pv_slice(pv, jj),
                                ones_bf_sb[0:1, 0:128],
                                bv_sb[0:1, :],
                                start=False, stop=True,
                            )
                        if jj % 2 == 1:
                            # both regions of this psum bank closed: scale them
                            # now so the Act/DVE work overlaps the next bank's
                            # matmuls
                            _v_finish1(pv, jc, jj - 1, on_act=on_act)
                            _v_finish1(pv, jc, jj, on_act=on_act)

                def gen_phaseK_rest():
                    for jc in range(1, NSKC):
                        if first and jc == NSKC - 1:
                            nc.sync.dma_start(wout_sb[:], wout.ap().rearrange("(t p) m -> p t m", p=128))
                        emit_K_kT(jc)
                        emit_K_v(jc, on_act=True)
                        yield

                # ====== Phase A: software-pipelined attention ======
                # Filler generators yield their approximate PE cost (ns) per
                # step so emit_filler hands the PE a time budget, not a count.
                MM = 213  # ns, 512-row bf16 matmul

                def gen_qproj(qc, qry_sb=None):
                    if qry_sb is None:
                        qry_sb = sp.tile([128, DK, SQC], BF, tag="strip", name="qry_sb")
                        nc.sync.dma_start(qry_sb[:], qryT_r[:, :, qc * SQC:(qc + 1) * SQC])
                        # let the DMA land before the first matmul is emitted,
                        # so the in-order PE doesn't stall on it
                        yield 0
                        yield 0
                        yield 0
                    yield 0
                    for cc in range(2):
                        pq = psp.tile([128, SQC], F32, tag="av", bufs=2, name="pq")
                        for d in range(DK):
                            nc.tensor.matmul(
                                pq[:],
                                wq_sb[:, d, cc * 128:(cc + 1) * 128],
                                qry_sb[:, d, :],
                                start=(d == 0), stop=(not with_bias and d == DK - 1),
                            )
                            yield MM
                        if with_bias:
                            nc.tensor.matmul(
                                pq[:],
                                bq_sb[0:1, cc * 128:(cc + 1) * 128],
                                ones_bf_sb[0:1, :],
                                start=False, stop=True,
                            )
                            yield MM
                        nc.vector.tensor_copy(
                            qt_all[:, cc, qc * SQC:(qc + 1) * SQC], pq[:]
                        )
                        yield 0

                def gen_outproj(qc, otn, epilogue=False, mrange=range(8)):
                    for m in mrange:
                        ptag = ("av", "mm")[m % 2] if epilogue else "av"
                        pf = psp.tile([128, SQC], F32, tag=ptag, bufs=2, name="pf")
                        nc.tensor.matmul(
                            pf[:],
                            wout_sb[:, 0, m * 128:(m + 1) * 128],
                            otn[:, 0, :],
                            start=True, stop=False,
                        )
                        yield MM
                        nc.tensor.matmul(
                            pf[:],
                            wout_sb[:, 1, m * 128:(m + 1) * 128],
                            otn[:, 1, :],
                            start=False, stop=True,
                        )
                        yield MM
                        fin = workp.tile([128, SQC], F32, tag="fin", bufs=8)
                        if epilogue and m % 2 == 0:
                            nc.scalar.copy(fin[:], pf[:])
                        else:
                            nc.vector.tensor_copy(fin[:], pf[:])
                        nc.sync.dma_start(
                            outT_r[:, m, qc * SQC:(qc + 1) * SQC], fin[:]
                        )
                        yield 0

                filler = []

                def emit_filler(budget_ns):
                    # emit filler steps until ~budget_ns of PE work was queued
                    while budget_ns > 0 and filler:
                        try:
                            budget_ns -= max(next(filler[0]), 40)
                        except StopIteration:
                            filler.pop(0)

                def gen_norm(pav, otn, pair, po):
                    # normalize as filler during the next head. The reciprocal
                    # reads the denominator row straight from PSUM so the pbc
                    # broadcast matmul trails the head's last AV by one DVE op,
                    # not the whole ot copy.
                    rcp = workp.tile([2, SQC], F32R, tag="rcp", bufs=2)
                    with nc.allow_low_precision(reason="fp32r reciprocal for softmax denom"):
                        nc.vector.reciprocal(rcp[0:1, :], pav[HD:HD + 1, :])
                    yield 0
                    ot = workp.tile([HD, SQC], F32, tag="ot", bufs=4)
                    nc.vector.tensor_copy(ot[:], pav[0:HD, :])
                    yield 0
                    pbc = psp.tile([128, SQC], F32, tag="av", bufs=2)
                    nc.tensor.matmul(
                        pbc[0:HD, :], ones_sb[0:1, 0:HD], rcp[0:1, :],
                        start=True, stop=True,
                    )
                    yield MM
                    nc.vector.tensor_mul(
                        otn[po:po + 64, pair, :], ot[:], pbc[0:HD, :]
                    )
                    yield 0

                emit_K_kT(0)
                # chunk 0's q-projection runs right after kT-jc0; v-jc0 is
                # emitted inside head 0's first attention group (scores need
                # only kT+qt; AV consumes v one group later), so the PE isn't
                # blocked on the wv DMA.
                for _ in gen_qproj(0, qry0_sb):
                    pass
                kgen = gen_phaseK_rest()

                GROUPS = (2, 3, 3, 3, 3, 2)
                kdone = [1]  # K-jc0 emitted in the prologue
                otn_hist = []  # [(qc, otn)] pending out-projections
                for qc in range(NSQC):
                    if qc + 1 < NSQC:
                        filler.append(gen_qproj(qc + 1))
                    if otn_hist:
                        filler.append(gen_outproj(otn_hist[-1][0], otn_hist[-1][1]))
                    qt = qt_all[:, :, qc * SQC:(qc + 1) * SQC]
                    otn = workp.tile([128, 2, SQC], BF, tag="otn", bufs=3)
                    for h in range(HG):
                        if qc == 0 and h == 1:
                            while kdone[0] < NSKC:
                                next(kgen)
                                kdone[0] += 1
                        pair, po = h // 2, (h % 2) * 64
                        pav = psp.tile([HD + 1, SQC], F32, tag="av", bufs=2)

                        def emit_av(prev, pav=None, h=None):
                            gs0, jbase0, pt0 = prev
                            for sub in range(gs0):
                                j = jbase0 + sub
                                nc.tensor.matmul(
                                    pav[:],
                                    v_sb[:, j, h, :],
                                    pt0[:, sub, :],
                                    start=(j == 0), stop=(j == NJ - 1),
                                )

                        # AV runs two groups behind scores, so the PE has a
                        # full Act-instruction of slack before it waits on exp.
                        avlag = 1 if (qc == 0 and h == 0) else 2
                        prevs = []
                        jbase = 0
                        for gi, gs in enumerate(GROUPS):
                            if qc == 0 and h == 0:
                                # emit K-jc sections one group ahead of need so
                                # the v-scale chain finishes before the AV that
                                # consumes it
                                need = min((jbase + gs + 1) // 4, NSKC - 1)
                                while kdone[0] <= need:
                                    next(kgen)
                                    kdone[0] += 1
                            ps = psp.tile([128, 3, SQC], F32, tag="mm", bufs=2)
                            for sub in range(gs):
                                j = jbase + sub
                                nc.tensor.matmul(
                                    ps[:, sub, :],
                                    kt_sb[po:po + 64, pair, j * 128:(j + 1) * 128],
                                    qt[po:po + 64, pair, :],
                                    start=True, stop=True,
                                )
                            pt = workp.tile([128, 3, SQC], BF, tag="pt", bufs=4)
                            nc.scalar.activation(pt[:, 0:gs, :], ps[:, 0:gs, :], EXP)
                            if qc == 0 and h == 0 and gi == 0:
                                emit_K_v(0)
                            prevs.append((gs, jbase, pt))
                            if len(prevs) > avlag:
                                emit_av(prevs.pop(0), pav=pav, h=h)
                            jbase += gs
                            if not (qc == 0 and h == 0):
                                emit_filler(400)
                        for prev in prevs:
                            emit_av(prev, pav=pav, h=h)
                        # normalize runs as priority filler during the next head
                        filler.insert(0, gen_norm(pav, otn, pair, po))
                    otn_hist.append((qc, otn))

                # ---- epilogue for the final chunk ----
                # The first-half matmuls of its out-projection only need otn
                # pair 0 (normalized long ago): emit 6 of them before draining
                # the filler so they cover the last heads' normalize chains.
                last_otn = otn_hist[-1][1]
                qcL = NSQC - 1
                pf3s = []
                for half in range(2):
                    pf3 = psp.tile([128, 3, SQC], F32, tag="mm", bufs=2, name="pfh")
                    for i in range(3):
                        m = half * 3 + i
                        nc.tensor.matmul(
                            pf3[:, i, :],
                            wout_sb[:, 0, m * 128:(m + 1) * 128],
                            last_otn[:, 0, :],
                            start=True, stop=False,
                        )
                    pf3s.append(pf3)
                emit_filler(10 ** 9)
                for half in range(2):
                    pf3 = pf3s[half]
                    for i in range(3):
                        m = half * 3 + i
                        nc.tensor.matmul(
                            pf3[:, i, :],
                            wout_sb[:, 1, m * 128:(m + 1) * 128],
                            last_otn[:, 1, :],
                            start=False, stop=True,
                        )
                        fin = workp.tile([128, SQC], F32, tag="fin", bufs=8)
                        if i == 1:
                            nc.scalar.copy(fin[:], pf3[:, i, :])
                        else:
                            nc.vector.tensor_copy(fin[:], pf3[:, i, :])
                        nc.sync.dma_start(outT_r[:, m, qcL * SQC:(qcL + 1) * SQC], fin[:])
                for m in (6, 7):
                    pf = psp.tile([128, SQC], F32, tag="av", bufs=2, name="pfe")
                    nc.tensor.matmul(
                        pf[:],
                        wout_sb[:, 0, m * 128:(m + 1) * 128],
                        last_otn[:, 0, :],
                        start=True, stop=False,
                    )
                    nc.tensor.matmul(
                        pf[:],
                        wout_sb[:, 1, m * 128:(m + 1) * 128],
                        last_otn[:, 1, :],
                        start=False, stop=True,
                    )
                    fin = workp.tile([128, SQC], F32, tag="fin", bufs=8)
                    if m == 6:
                        nc.scalar.copy(fin[:], pf[:])
                    else:
                        nc.vector.tensor_copy(fin[:], pf[:])
                    nc.sync.dma_start(outT_r[:, m, qcL * SQC:(qcL + 1) * SQC], fin[:])

            for p in range(passes):
                run_pass(p == 0)

    nc.compile()
    return nc


def _get_nc(with_bias=False, passes=1):
    key = f"nc{int(with_bias)}p{passes}"
    if key not in _CACHE:
        _CACHE[key] = _build(with_bias, passes)
    return _CACHE[key]


LAST_RESULTS = None
LAST_IN_MAPS = None


def _bf16(x: np.ndarray) -> np.ndarray:
    import ml_dtypes

    return np.ascontiguousarray(np.asarray(x, dtype=np.float32)).astype(
        ml_dtypes.bfloat16
    )


def kernel(query, context, mask, Wq, bq, Wkv, bkv, Wout, bout, num_heads):
    import os
    from concourse.bass_utils import run_bass_kernel_spmd

    query = np.asarray(query, dtype=np.float32)
    context = np.asarray(context, dtype=np.float32)
    mask = np.asarray(mask)
    Wq = np.asarray(Wq, dtype=np.float32)
    bq_v = np.asarray(bq, dtype=np.float32)
    Wkv = np.asarray(Wkv, dtype=np.float32)
    bkv_v = np.asarray(bkv, dtype=np.float32)
    Wout = np.asarray(Wout, dtype=np.float32)
    bout_v = np.asarray(bout, dtype=np.float32)
    assert int(num_heads) == H

    scale = np.float32(HD ** -0.5)
    Wq_s = Wq * scale
    bq_s = bq_v * scale
    Wk = Wkv[:, :D]
    Wv = Wkv[:, D:]
    bk_v = bkv_v[:D]
    bv_v = bkv_v[D:]
    keep_f = 1.0 - mask.astype(np.float32)          # [B, SKV]
    ones_r = np.ones((1, SQC), dtype=np.float32)
    ones2_r = np.zeros((2, 128), dtype=np.float32)
    ones2_r[0, :64] = 1.0
    ones2_r[1, 64:] = 1.0

    with_bias = bool(np.any(bq_s) or np.any(bk_v) or np.any(bv_v))
    nc = _get_nc(with_bias)
    in_maps = []
    for c in range(8):
        b, g = c // 4, c % 4
        cs = slice(g * COLS, (g + 1) * COLS)
        in_maps.append({
            "qryT": _bf16(query[b].T),
            "ctxT": _bf16(context[b].T),
            "wq": _bf16(Wq_s[:, cs]),
            "wk": _bf16(Wk[:, cs]),
            "wv": _bf16(Wv[:, cs]),
            "wout": _bf16(Wout[cs, :]),
            "bq": _bf16(bq_s[cs][None, :]),
            "bk": _bf16(bk_v[cs][None, :]),
            "bv": _bf16(bv_v[cs][None, :]),
            "ones": ones_r,
            "ones2": ones2_r,
            "ones_bf": _bf16(ones_r),
            "keep": np.ascontiguousarray(keep_f[b].reshape(NJ, 128).T),
            "keep_rep": np.ascontiguousarray(
                np.repeat(keep_f[b].reshape(NJ, 128).T, HG, axis=1)
            ),
        })

    trace = bool(int(os.environ.get("KERNEL_TRACE", "0")))
    res = run_bass_kernel_spmd(nc, in_maps, core_ids=list(range(8)), trace=trace)
    global LAST_RESULTS, LAST_IN_MAPS
    LAST_RESULTS = res
    LAST_IN_MAPS = in_maps

    out = np.empty((B, SQ, D), dtype=np.float32)
    for b in range(B):
        acc = np.zeros((D, SQ), dtype=np.float32)
        for g in range(4):
            acc += res.results[b * 4 + g]["outT"]
        out[b] = acc.T + bout_v[None, :]
    return out


# revision 80
# speedup vs baseline: 1.0027x; 1.0027x over previous
"""Cross-attention Trainium2 kernel (8 NeuronCores, Bass/Tile).

Problem (hardcoded): B=2, SQ=SKV=2048, D=1024, H=16 heads, HD=64.
  q  = query @ Wq + bq
  kv = context @ Wkv + bkv ; split into k, v per head
  o  = softmax(q k^T / sqrt(hd) + mask) v         (mask: -inf where True)
  out = o @ Wout + bout

Sharding: core c = (b, g) with b = c // 4 (batch), g = c % 4 (head group of 4).
Each core computes its batch's attention for its 4 heads and the partial out
projection (Wout rows for those heads); host sums the 4 partials per batch and
adds bout (linearity of the out projection).

Everything on-chip runs "transposed" (feature dim on partitions, tokens on the
free dim), so the host passes query/context transposed and gets the partial
output transposed back. Softmax uses no max subtraction (scores are ~N(0,1)
here; exp is safe in fp32) and folds masking into V: v rows are scaled by
keep=1-mask and an extra "keep" column of V yields the softmax denominator via
the same PE accumulation.

Inputs and all activation tiles are bfloat16 (within the rel-err budget; the
PE streams bf16 at the same 1 col/cycle as fp32r but DMA traffic halves, which
un-bottlenecks the startup phase where ~6 MB must land before the first
attention chunk completes). PSUM accumulation, softmax reciprocal, and the
output partials stay fp32/fp32r.

NOTE (hardware-validated): PSUM accumulation groups must be emitted
contiguously per region — interleaving two open start/stop accumulation
sequences that share a PSUM bank silently corrupts results on HW (CoreSim
does not catch it).

`passes` emits the whole body N times into one NEFF (weights loaded once) —
used by test.py to measure one steady-state iteration differentially.
"""

import sys

sys.path.insert(0, "/opt/trn_rl_repo")

import numpy as np

B, SQ, SKV, D, H, HD = 2, 2048, 2048, 1024, 16, 64
HG = 4                # heads per core
COLS = HG * HD        # 256 projected columns per core (per q/k/v)
DK = D // 128         # 8 contraction tiles
SQC = 512             # sq chunk (psum bank)
NSQC = SQ // SQC
SKC = 512             # skv chunk for kv projection
NSKC = SKV // SKC
NJ = SKV // 128       # 16 skv tiles for attention

_CACHE = {}


def _build(with_bias=False, passes=1):
    import concourse.bacc as bacc
    import concourse.mybir as mybir
    import concourse.tile as tile

    F32 = mybir.dt.float32
    F32R = mybir.dt.float32r
    BF = mybir.dt.bfloat16
    EXP = mybir.ActivationFunctionType.Exp

    nc = bacc.Bacc()

    # ---- DRAM I/O (per core) ----
    qryT = nc.dram_tensor("qryT", [D, SQ], BF, kind="ExternalInput")
    ctxT = nc.dram_tensor("ctxT", [D, SKV], BF, kind="ExternalInput")
    wq = nc.dram_tensor("wq", [D, COLS], BF, kind="ExternalInput")
    wk = nc.dram_tensor("wk", [D, COLS], BF, kind="ExternalInput")
    wv = nc.dram_tensor("wv", [D, COLS], BF, kind="ExternalInput")
    wout = nc.dram_tensor("wout", [COLS, D], BF, kind="ExternalInput")
    bq = nc.dram_tensor("bq", [1, COLS], BF, kind="ExternalInput")
    bk = nc.dram_tensor("bk", [1, COLS], BF, kind="ExternalInput")
    bv = nc.dram_tensor("bv", [1, COLS], BF, kind="ExternalInput")
    ones = nc.dram_tensor("ones", [1, SQC], F32R, kind="ExternalInput")
    ones2 = nc.dram_tensor("ones2", [2, 128], F32R, kind="ExternalInput")
    ones_bf = nc.dram_tensor("ones_bf", [1, SQC], BF, kind="ExternalInput")
    keep = nc.dram_tensor("keep", [128, NJ], F32, kind="ExternalInput")
    keep_rep = nc.dram_tensor("keep_rep", [128, NJ * HG], F32, kind="ExternalInput")
    outT = nc.dram_tensor("outT", [D, SQ], F32, kind="ExternalOutput")

    with tile.TileContext(nc) as tc:
        with (
            tc.tile_pool(name="w", bufs=1) as wp,
            tc.tile_pool(name="big", bufs=1) as bigp,
            tc.tile_pool(name="strips", bufs=3) as sp,
            tc.tile_pool(name="work", bufs=1) as workp,
            tc.tile_pool(name="ps", bufs=1, space="PSUM") as psp,
        ):
            # ---- weights / constants (loaded once) ----
            wq_sb = wp.tile([128, DK, COLS], BF)
            wk_sb = wp.tile([128, DK, COLS], BF)
            wv_sb = wp.tile([128, DK, COLS], BF)
            wout_sb = wp.tile([128, 2, D], BF)
            bq_sb = wp.tile([1, COLS], BF)
            bk_sb = wp.tile([1, COLS], BF)
            bv_sb = wp.tile([1, COLS], BF)
            ones_sb = wp.tile([1, SQC], F32R)
            ones_bf_sb = wp.tile([1, SQC], BF)
            keep_sb = wp.tile([128, NJ], F32)
            keep_rep_sb = wp.tile([128, NJ, HG], F32)

            ctxT_r = ctxT.ap().rearrange("(t p) s -> p t s", p=128)
            qryT_r = qryT.ap().rearrange("(t p) s -> p t s", p=128)
            outT_r = outT.ap().rearrange("(t p) s -> p t s", p=128)
            wk_r = wk.ap().rearrange("(t p) m -> p t m", p=128)
            wq_r = wq.ap().rearrange("(t p) m -> p t m", p=128)

            # ---- persistent activations ----
            kt_sb = bigp.tile([128, 2, SKV], BF)          # k^T, head pair per 64-row band
            v_sb = bigp.tile([128, NJ, HG, HD + 1], BF)   # v + keep column, [skv%128, j, h, :]
            qt_all = bigp.tile([128, 2, SQ], BF)          # q^T for all chunks

            def run_pass(first):
                # DMA granularity: the HWDGE costs ~625ns per descriptor
                # (serialized), so each DMA should carry >=~0.25 MB. The
                # startup-critical wk/ctx0 pair is split into 3 chunks each
                # (interleaved in consumption order) so the first kT matmul
                # starts after ~0.4 MB, not 1 MB.
                ctx0_sb = sp.tile([128, DK, SKC], BF, tag="strip", name="ctx0_sb")
                for dlo, dhi in ((0, 2), (2, 5), (5, 8)):
                    if first:
                        nc.sync.dma_start(wk_sb[:, dlo:dhi, :], wk_r[:, dlo:dhi, :])
                    nc.sync.dma_start(ctx0_sb[:, dlo:dhi, :], ctxT_r[:, dlo:dhi, 0:SKC])
                # qproj(0) runs between kT-jc0 and v-jc0 on the PE, so its
                # inputs (qry0, wq) come right after the kT inputs.
                qry0_sb = sp.tile([128, DK, SQC], BF, tag="strip", name="qry0_sb")
                for dlo, dhi in ((0, 4), (4, 8)):
                    if first:
                        nc.sync.dma_start(wq_sb[:, dlo:dhi, :], wq_r[:, dlo:dhi, :])
                    nc.sync.dma_start(qry0_sb[:, dlo:dhi, :], qryT_r[:, dlo:dhi, 0:SQC])
                if first:
                    nc.sync.dma_start(wv_sb[:], wv.ap().rearrange("(t p) m -> p t m", p=128))
                    nc.sync.dma_start(keep_sb[:], keep.ap())
                    nc.sync.dma_start(
                        keep_rep_sb[:],
                        keep_rep.ap().rearrange("p (j h) -> p j h", h=HG),
                    )
                    if with_bias:
                        nc.sync.dma_start(ones_bf_sb[:], ones_bf.ap())
                        nc.sync.dma_start(bk_sb[:], bk.ap())
                        nc.sync.dma_start(bq_sb[:], bq.ap())
                        nc.sync.dma_start(bv_sb[:], bv.ap())
                # pre-issue the remaining ctx strips so they queue ahead of
                # wout and the later qry strips; the small keep_rep/ones loads
                # slot in after the strip they must not delay
                strip_tiles = [ctx0_sb]
                for jc in range(1, NSKC):
                    st = sp.tile([128, DK, SKC], BF, tag="strip", name=f"ctx{jc}_sb")
                    nc.sync.dma_start(st[:], ctxT_r[:, :, jc * SKC:(jc + 1) * SKC])
                    strip_tiles.append(st)
                    if first and jc == 2:
                        nc.sync.dma_start(ones_sb[:], ones.ap())

                if first:
                    # the keep column of v is constant: fill it for all chunks
                    # in one op
                    nc.vector.tensor_copy(v_sb[:, :, :, HD:HD + 1], keep_rep_sb[:])

                # ===== Phase K (kv projection), interleaved into head 0 =====
                def emit_K_kT(jc):
                    ctx_sb = strip_tiles[jc]
                    pk = psp.tile([128, 2, SKC], F32, tag="mm", bufs=2, name="pk")
                    for cc in range(2):
                        for d in range(DK):
                            nc.tensor.matmul(
                                pk[:, cc, :],
                                wk_sb[:, d, cc * 128:(cc + 1) * 128],
                                ctx_sb[:, d, :],
                                start=(d == 0), stop=(not with_bias and d == DK - 1),
                            )
                        if with_bias:
                            nc.tensor.matmul(
                                pk[:, cc, :],
                                bk_sb[0:1, cc * 128:(cc + 1) * 128],
                                ones_bf_sb[0:1, :],
                                start=False, stop=True,
                            )
                    nc.vector.tensor_copy(kt_sb[:, :, jc * SKC:(jc + 1) * SKC], pk[:])

                def _pv_slice(pv, jj):
                    return pv[:, jj // 2, (jj % 2) * COLS:(jj % 2) * COLS + COLS]

                def _v_finish1(pv, jc, jj, on_act=False):
                    # on_act: run the scale on the Activation engine (idle
                    # during phase K) so it overlaps the DVE kt copy — the AV
                    # group that consumes this chunk's v tiles is only ~2us out.
                    j = jc * 4 + jj
                    src = _pv_slice(pv, jj).rearrange("p (h e) -> p h e", h=HG)
                    if on_act:
                        nc.scalar.mul(v_sb[:, j, :, 0:HD], src, keep_sb[:, j:j + 1])
                    else:
                        nc.vector.tensor_scalar_mul(
                            v_sb[:, j, :, 0:HD], src, keep_sb[:, j:j + 1]
                        )

                def emit_K_v(jc, on_act=False):
                    # each jj region's start..stop matmuls stay contiguous
                    # (HW requirement); its scale is emitted right after so the
                    # Act/DVE work overlaps the next region's matmuls
                    ctx_sb = strip_tiles[jc]
                    pv = psp.tile([128, 2, SKC], F32, tag="mm", bufs=2, name="pv")
                    for jj in range(4):
                        for d in range(DK):
                            nc.tensor.matmul(
                                _pv_slice(pv, jj),
                                ctx_sb[:, d, jj * 128:(jj + 1) * 128],
                                wv_sb[:, d, :],
                                start=(d == 0), stop=(not with_bias and d == DK - 1),
                            )
                        if with_bias:
                            nc.tensor.matmul(
                                _pv_slice(pv, jj),
                                ones_bf_sb[0:1, 0:128],
                                bv_sb[0:1, :],
                                start=False, stop=True,
                            )
                        if jj % 2 == 1:
                            # both regions of this psum bank closed: scale them
                            # now so the Act/DVE work overlaps the next bank's
                            # matmuls
                            _v_finish1(pv, jc, jj - 1, on_act=on_act)
                            _v_finish1(pv, jc, jj, on_act=on_act)

                def gen_phaseK_rest():
                    for jc in range(1, NSKC):
                        if first and jc == NSKC - 1:
                            nc.sync.dma_start(wout_sb[:], wout.ap().rearrange("(t p) m -> p t m", p=128))
                        emit_K_kT(jc)
                        emit_K_v(jc, on_act=True)
                        yield

                # ====== Phase A: software-pipelined attention ======
                # Filler generators yield their approximate PE cost (ns) per
                # step so emit_filler hands the PE a time budget, not a count.
                MM = 213  # ns, 512-row bf16 matmul

                def gen_qproj(qc, qry_sb=None):
                    if qry_sb is None:
                        qry_sb = sp.tile([128, DK, SQC], BF, tag="strip", name="qry_sb")
                        nc.sync.dma_start(qry_sb[:], qryT_r[:, :, qc * SQC:(qc + 1) * SQC])
                        # let the DMA land before the first matmul is emitted,
                        # so the in-order PE doesn't stall on it
                        yield 0
                        yield 0
                        yield 0
                    yield 0
                    for cc in range(2):
                        pq = psp.tile([128, SQC], F32, tag="av", bufs=2, name="pq")
                        for d in range(DK):
                            nc.tensor.matmul(
                                pq[:],
                                wq_sb[:, d, cc * 128:(cc + 1) * 128],
                                qry_sb[:, d, :],
                                start=(d == 0), stop=(not with_bias and d == DK - 1),
                            )
                            yield MM
                        if with_bias:
                            nc.tensor.matmul(
                                pq[:],
                                bq_sb[0:1, cc * 128:(cc + 1) * 128],
                                ones_bf_sb[0:1, :],
                                start=False, stop=True,
                            )
                            yield MM
                        nc.vector.tensor_copy(
                            qt_all[:, cc, qc * SQC:(qc + 1) * SQC], pq[:]
                        )
                        yield 0

                def gen_outproj(qc, otn, epilogue=False, mrange=range(8)):
                    for m in mrange:
                        ptag = ("av", "mm")[m % 2] if epilogue else "av"
                        pf = psp.tile([128, SQC], F32, tag=ptag, bufs=2, name="pf")
                        nc.tensor.matmul(
                            pf[:],
                            wout_sb[:, 0, m * 128:(m + 1) * 128],
                            otn[:, 0, :],
                            start=True, stop=False,
                        )
                        yield MM
                        nc.tensor.matmul(
                            pf[:],
                            wout_sb[:, 1, m * 128:(m + 1) * 128],
                            otn[:, 1, :],
                            start=False, stop=True,
                        )
                        yield MM
                        fin = workp.tile([128, SQC], F32, tag="fin", bufs=8)
                        if epilogue and m % 2 == 0:
                            nc.scalar.copy(fin[:], pf[:])
                        else:
                            nc.vector.tensor_copy(fin[:], pf[:])
                        nc.sync.dma_start(
                            outT_r[:, m, qc * SQC:(qc + 1) * SQC], fin[:]
                        )
                        yield 0

                filler = []

                def emit_filler(budget_ns):
                    # emit filler steps until ~budget_ns of PE work was queued
                    while budget_ns > 0 and filler:
                        try:
                            budget_ns -= max(next(filler[0]), 40)
                        except StopIteration:
                            filler.pop(0)

                def gen_norm(pav, otn, pair, po, ot_on_act=False):
                    # normalize as filler during the next head. The reciprocal
                    # reads the denominator row straight from PSUM so the pbc
                    # broadcast matmul trails the head's last AV by one DVE op,
                    # not the whole ot copy. For the final head the ot copy
                    # runs on Act (idle by then) so the DVE chain is rcp->mul.
                    rcp = workp.tile([2, SQC], F32R, tag="rcp", bufs=2)
                    with nc.allow_low_precision(reason="fp32r reciprocal for softmax denom"):
                        nc.vector.reciprocal(rcp[0:1, :], pav[HD:HD + 1, :])
                    yield 0
                    ot = workp.tile([HD, SQC], F32, tag="ot", bufs=4)
                    if ot_on_act:
                        nc.scalar.copy(ot[:], pav[0:HD, :])
                    else:
                        nc.vector.tensor_copy(ot[:], pav[0:HD, :])
                    yield 0
                    pbc = psp.tile([128, SQC], F32, tag="av", bufs=2)
                    nc.tensor.matmul(
                        pbc[0:HD, :], ones_sb[0:1, 0:HD], rcp[0:1, :],
                        start=True, stop=True,
                    )
                    yield MM
                    nc.vector.tensor_mul(
                        otn[po:po + 64, pair, :], ot[:], pbc[0:HD, :]
                    )
                    yield 0

                emit_K_kT(0)
                # chunk 0's q-projection runs right after kT-jc0; v-jc0 is
                # emitted inside head 0's first attention group (scores need
                # only kT+qt; AV consumes v one group later), so the PE isn't
                # blocked on the wv DMA.
                for _ in gen_qproj(0, qry0_sb):
                    pass
                kgen = gen_phaseK_rest()

                GROUPS = (2, 3, 3, 3, 3, 2)
                kdone = [1]  # K-jc0 emitted in the prologue
                otn_hist = []  # [(qc, otn)] pending out-projections
                for qc in range(NSQC):
                    if qc + 1 < NSQC:
                        filler.append(gen_qproj(qc + 1))
                    if otn_hist:
                        filler.append(gen_outproj(otn_hist[-1][0], otn_hist[-1][1]))
                    qt = qt_all[:, :, qc * SQC:(qc + 1) * SQC]
                    otn = workp.tile([128, 2, SQC], BF, tag="otn", bufs=3)
                    for h in range(HG):
                        if qc == 0 and h == 1:
                            while kdone[0] < NSKC:
                                next(kgen)
                                kdone[0] += 1
                        pair, po = h // 2, (h % 2) * 64
                        pav = psp.tile([HD + 1, SQC], F32, tag="av", bufs=2)

                        def emit_av(prev, pav=None, h=None):
                            gs0, jbase0, pt0 = prev
                            for sub in range(gs0):
                                j = jbase0 + sub
                                nc.tensor.matmul(
                                    pav[:],
                                    v_sb[:, j, h, :],
                                    pt0[:, sub, :],
                                    start=(j == 0), stop=(j == NJ - 1),
                                )

                        # AV runs two groups behind scores, so the PE has a
                        # full Act-instruction of slack before it waits on exp.
                        avlag = 1 if (qc == 0 and h == 0) else 2
                        prevs = []
                        jbase = 0
                        for gi, gs in enumerate(GROUPS):
                            if qc == 0 and h == 0:
                                # emit K-jc sections one group ahead of need so
                                # the v-scale chain finishes before the AV that
                                # consumes it
                                need = min((jbase + gs + 1) // 4, NSKC - 1)
                                while kdone[0] <= need:
                                    next(kgen)
                                    kdone[0] += 1
                            ps = psp.tile([128, 3, SQC], F32, tag="mm", bufs=2)
                            for sub in range(gs):
                                j = jbase + sub
                                nc.tensor.matmul(
                                    ps[:, sub, :],
                                    kt_sb[po:po + 64, pair, j * 128:(j + 1) * 128],
                                    qt[po:po + 64, pair, :],
                                    start=True, stop=True,
                                )
                            pt = workp.tile([128, 3, SQC], BF, tag="pt", bufs=4)
                            nc.scalar.activation(pt[:, 0:gs, :], ps[:, 0:gs, :], EXP)
                            if qc == 0 and h == 0 and gi == 0:
                                emit_K_v(0)
                            prevs.append((gs, jbase, pt))
                            if len(prevs) > avlag:
                                emit_av(prevs.pop(0), pav=pav, h=h)
                            jbase += gs
                            if not (qc == 0 and h == 0):
                                # late chunks have less filler supply: throttle
                                # the per-group budget so it lasts to the end
                                # of the chunk instead of running dry at h2/h3
                                emit_filler((400, 400, 350, 170)[qc])
                        for prev in prevs:
                            emit_av(prev, pav=pav, h=h)
                        # normalize runs as priority filler during the next head
                        filler.insert(0, gen_norm(pav, otn, pair, po))
                    otn_hist.append((qc, otn))

                # ---- epilogue for the final chunk ----
                # The first-half matmuls of its out-projection only need otn
                # pair 0 (normalized long ago): emit 6 of them before draining
                # the filler so they cover the last heads' normalize chains.
                last_otn = otn_hist[-1][1]
                qcL = NSQC - 1
                pf3s = []
                for half in range(2):
                    pf3 = psp.tile([128, 3, SQC], F32, tag="mm", bufs=2, name="pfh")
                    for i in range(3):
                        m = half * 3 + i
                        nc.tensor.matmul(
                            pf3[:, i, :],
                            wout_sb[:, 0, m * 128:(m + 1) * 128],
                            last_otn[:, 0, :],
                            start=True, stop=False,
                        )
                    pf3s.append(pf3)
                emit_filler(10 ** 9)
                for half in range(2):
                    pf3 = pf3s[half]
                    for i in range(3):
                        m = half * 3 + i
                        nc.tensor.matmul(
                            pf3[:, i, :],
                            wout_sb[:, 1, m * 128:(m + 1) * 128],
                            last_otn[:, 1, :],
                            start=False, stop=True,
                        )
                        fin = workp.tile([128, SQC], F32, tag="fin", bufs=8)
                        if i == 1:
                            nc.scalar.copy(fin[:], pf3[:, i, :])
                        else:
                            nc.vector.tensor_copy(fin[:], pf3[:, i, :])
                        nc.sync.dma_start(outT_r[:, m, qcL * SQC:(qcL + 1) * SQC], fin[:])
                for m in (6, 7):
                    pf = psp.tile([128, SQC], F32, tag="av", bufs=2, name="pfe")
                    nc.tensor.matmul(
                        pf[:],
                        wout_sb[:, 0, m * 128:(m + 1) * 128],
                        last_otn[:, 0, :],
                        start=True, stop=False,
                    )
                    nc.tensor.matmul(
                        pf[:],
                        wout_sb[:, 1, m * 128:(m + 1) * 128],
                        last_otn[:, 1, :],
                        start=False, stop=True,
                    )
                    fin = workp.tile([128, SQC], F32, tag="fin", bufs=8)
                    if m == 6:
                        nc.scalar.copy(fin[:], pf[:])
                    else:
                        nc.vector.tensor_copy(fin[:], pf[:])
                    nc.sync.dma_start(outT_r[:, m, qcL * SQC:(qcL + 1) * SQC], fin[:])

            for p in range(passes):
                run_pass(p == 0)

    nc.compile()
    return nc


def _get_nc(with_bias=False, passes=1):
    key = f"nc{int(with_bias)}p{passes}"
    if key not in _CACHE:
        _CACHE[key] = _build(with_bias, passes)
    return _CACHE[key]


LAST_RESULTS = None
LAST_IN_MAPS = None


def _bf16(x: np.ndarray) -> np.ndarray:
    import ml_dtypes

    return np.ascontiguousarray(np.asarray(x, dtype=np.float32)).astype(
        ml_dtypes.bfloat16
    )


def kernel(query, context, mask, Wq, bq, Wkv, bkv, Wout, bout, num_heads):
    import os
    from concourse.bass_utils import run_bass_kernel_spmd

    query = np.asarray(query, dtype=np.float32)
    context = np.asarray(context, dtype=np.float32)
    mask = np.asarray(mask)
    Wq = np.asarray(Wq, dtype=np.float32)
    bq_v = np.asarray(bq, dtype=np.float32)
    Wkv = np.asarray(Wkv, dtype=np.float32)
    bkv_v = np.asarray(bkv, dtype=np.float32)
    Wout = np.asarray(Wout, dtype=np.float32)
    bout_v = np.asarray(bout, dtype=np.float32)
    assert int(num_heads) == H

    scale = np.float32(HD ** -0.5)
    Wq_s = Wq * scale
    bq_s = bq_v * scale
    Wk = Wkv[:, :D]
    Wv = Wkv[:, D:]
    bk_v = bkv_v[:D]
    bv_v = bkv_v[D:]
    keep_f = 1.0 - mask.astype(np.float32)          # [B, SKV]
    ones_r = np.ones((1, SQC), dtype=np.float32)
    ones2_r = np.zeros((2, 128), dtype=np.float32)
    ones2_r[0, :64] = 1.0
    ones2_r[1, 64:] = 1.0

    with_bias = bool(np.any(bq_s) or np.any(bk_v) or np.any(bv_v))
    nc = _get_nc(with_bias)
    in_maps = []
    for c in range(8):
        b, g = c // 4, c % 4
        cs = slice(g * COLS, (g + 1) * COLS)
        in_maps.append({
            "qryT": _bf16(query[b].T),
            "ctxT": _bf16(context[b].T),
            "wq": _bf16(Wq_s[:, cs]),
            "wk": _bf16(Wk[:, cs]),
            "wv": _bf16(Wv[:, cs]),
            "wout": _bf16(Wout[cs, :]),
            "bq": _bf16(bq_s[cs][None, :]),
            "bk": _bf16(bk_v[cs][None, :]),
            "bv": _bf16(bv_v[cs][None, :]),
            "ones": ones_r,
            "ones2": ones2_r,
            "ones_bf": _bf16(ones_r),
            "keep": np.ascontiguousarray(keep_f[b].reshape(NJ, 128).T),
            "keep_rep": np.ascontiguousarray(
                np.repeat(keep_f[b].reshape(NJ, 128).T, HG, axis=1)
            ),
        })

    trace = bool(int(os.environ.get("KERNEL_TRACE", "0")))
    res = run_bass_kernel_spmd(nc, in_maps, core_ids=list(range(8)), trace=trace)
    global LAST_RESULTS, LAST_IN_MAPS
    LAST_RESULTS = res
    LAST_IN_MAPS = in_maps

    out = np.empty((B, SQ, D), dtype=np.float32)
    for b in range(B):
        acc = np.zeros((D, SQ), dtype=np.float32)
        for g in range(4):
            acc += res.results[b * 4 + g]["outT"]
        out[b] = acc.T + bout_v[None, :]
    return out
